# revision 12
# baseline (speedup 1.0000x reference)
"""Trainium2 Bass kernel for nn_CurvedMultiHeadAttention (B=4, S=1024, E=768, H=12, D=64, R=16).

Sharding: 8 cores; core c handles batch b=c//2 and heads h0=6*(c%2) .. h0+5.
Each core computes its 6 heads' out-projection contribution (bo/2 folded in);
the host sums the two partials per batch element.

Math (validated ~1e-6 vs reference in fp64):
 - softmax over keys is invariant to per-query shifts => qq term drops.
 - EPS*I part of G_h contributes <1e-5 to scores => dropped.
 - Weff_q = A_h^T Wq_h (16x768 per head) is folded on the HOST (weight-only
   transform), so the device projects hidden straight to qA/kA (16 dims/head,
   padded to 32 for PE quadrant alignment).
 - q/k biases fold into the per-key exp bias:
     exp arg = ESC*(kA0.qA0) + ckk[t],
     ckk = -SCALE*|kA0|^2 + kA0.(ESC*(bqA-bkA)) + mask
   computed with two small indicator matmuls (ksq x ind + kA x bvec).
 - softmax denominator comes free as a ones column appended to v in the ctx
   matmul; normalization happens during PSUM evacuation (per-partition scalar).

Schedule: weights-first multi-queue DMA; qk projection (PE) streams behind the
hT chunk DMAs; exp (ScalarE) starts ~16us in and runs back-to-back; v
projection, ctx, transposes and out-projection j-chunks fill PE slack under
the exp stream; 2-wave out-projection tail.
"""

import os
import numpy as np
import ml_dtypes

import concourse.bass as bass
import concourse.tile as tile
from concourse import bacc
from concourse import mybir
from concourse.bass_utils import run_bass_kernel_spmd

F32 = mybir.dt.float32
BF16 = mybir.dt.bfloat16
AF = mybir.ActivationFunctionType
ALU = mybir.AluOpType

S = 1024          # sequence length
E = 768           # embed
D = 64            # head dim
R = 16            # rank
HPC = 6           # heads per core
NCORES = 8
SCALE = 1.0 / 8.0
ESC = 2.0 * SCALE  # exp scale
NCH = 6           # contraction chunks of E

LAST_RESULTS = None     # BassKernelResults of the most recent run (for test.py)


def _emit(tc):
    nc = tc.nc
    hTd = nc.dram_tensor("hTa", [E, S], BF16, kind="ExternalInput")
    wQd = nc.dram_tensor("weffQ", [128, NCH * 128], BF16, kind="ExternalInput")
    wKd = nc.dram_tensor("weffK", [128, NCH * 128], BF16, kind="ExternalInput")
    wQ2d = nc.dram_tensor("weffQ2", [128, NCH * 128], BF16, kind="ExternalInput")
    wK2d = nc.dram_tensor("weffK2", [128, NCH * 128], BF16, kind="ExternalInput")
    wvd = nc.dram_tensor("WvTa", [E, HPC * D], BF16, kind="ExternalInput")
    wod = nc.dram_tensor("WoT", [HPC * D, E], BF16, kind="ExternalInput")
    mkd = nc.dram_tensor("maskT48", [128, 48], F32, kind="ExternalInput")
    indKd = nc.dram_tensor("indK", [128, 6], BF16, kind="ExternalInput")
    bvecKd = nc.dram_tensor("bvecK", [128, 6], BF16, kind="ExternalInput")
    indMd = nc.dram_tensor("indM2", [128, 6], BF16, kind="ExternalInput")
    bvecMd = nc.dram_tensor("bvecM2", [128, 6], BF16, kind="ExternalInput")
    bvd = nc.dram_tensor("bv_bc", [128, HPC * D], F32, kind="ExternalInput")
    bod = nc.dram_tensor("bo2", [128, E], F32, kind="ExternalInput")
    idd = nc.dram_tensor("ident", [128, 128], BF16, kind="ExternalInput")
    outd = nc.dram_tensor("outp", [S, E], F32, kind="ExternalOutput")

    import contextlib
    stack = contextlib.ExitStack()
    const = stack.enter_context(tc.tile_pool(name="const", bufs=1))
    work = stack.enter_context(tc.tile_pool(name="work", bufs=4))
    ptp = stack.enter_context(tc.tile_pool(name="ptp", bufs=1))
    psp = stack.enter_context(tc.tile_pool(name="psp", bufs=2, space="PSUM"))

    def psA():
        return psp.tile([128, 1024], F32, name="psA", tag="psA", bufs=3)

    def psC():
        return psp.tile([128, 512], F32, name="psC", tag="psC", bufs=2)

    cp = nc.vector.tensor_copy

    # ---------------- DMA: weights first, two queues ----------------
    # sync queue: weff chunks, then small consts, wv, wo, bo
    wKt = const.tile([128, NCH * 128], BF16, name="wKt", tag="wKt")
    nc.sync.dma_start(out=wKt[:, :], in_=wKd[:, :])
    wK2t = const.tile([128, NCH * 128], BF16, name="wK2t", tag="wK2t")
    nc.sync.dma_start(out=wK2t[:, :], in_=wK2d[:, :])
    wQt = const.tile([128, NCH * 128], BF16, name="wQt", tag="wQt")
    nc.sync.dma_start(out=wQt[:, :], in_=wQd[:, :])
    wQ2t = const.tile([128, NCH * 128], BF16, name="wQ2t", tag="wQ2t")
    nc.sync.dma_start(out=wQ2t[:, :], in_=wQ2d[:, :])
    wQ = [wQt[:, 128 * c:128 * (c + 1)] for c in range(NCH)]
    wK = [wKt[:, 128 * c:128 * (c + 1)] for c in range(NCH)]
    wQ2 = [wQ2t[:, 128 * c:128 * (c + 1)] for c in range(NCH)]
    wK2 = [wK2t[:, 128 * c:128 * (c + 1)] for c in range(NCH)]
    maskT = const.tile([128, 48], F32, name="maskT", tag="maskT")
    nc.sync.dma_start(out=maskT[:, :], in_=mkd[:, :])
    indK = const.tile([128, 6], BF16, name="indK", tag="indK")
    nc.sync.dma_start(out=indK[:, :], in_=indKd[:, :])
    bvecK = const.tile([128, 6], BF16, name="bvecK", tag="bvecK")
    nc.sync.dma_start(out=bvecK[:, :], in_=bvecKd[:, :])
    indM = const.tile([128, 6], BF16, name="indM", tag="indM")
    nc.sync.dma_start(out=indM[:, :], in_=indMd[:, :])
    bvecM = const.tile([128, 6], BF16, name="bvecM", tag="bvecM")
    nc.sync.dma_start(out=bvecM[:, :], in_=bvecMd[:, :])
    wv = []
    for c in range(NCH):
        wv.append(const.tile([128, HPC * D], BF16, name=f"wv{c}", tag=f"wv{c}"))
        nc.sync.dma_start(out=wv[c][:, :], in_=wvd[128 * c:128 * (c + 1), :])
    bv_bc = const.tile([128, HPC * D], F32, name="bv_bc", tag="bv_bc")
    nc.sync.dma_start(out=bv_bc[:, :], in_=bvd[:, :])
    ident = const.tile([128, 128], BF16, name="ident", tag="ident")
    nc.sync.dma_start(out=ident[:, :], in_=idd[:, :])
    wo = []
    for j in range(3):
        wo.append(const.tile([128, E], BF16, name=f"wo{j}", tag=f"wo{j}"))
        nc.sync.dma_start(out=wo[j][:, :], in_=wod[128 * j:128 * (j + 1), :])
    bo_bc = const.tile([128, E], F32, name="bo_bc", tag="bo_bc")
    nc.sync.dma_start(out=bo_bc[:, :], in_=bod[:, :])

    # gpsimd queue: hT chunks (critical path for qk projection)
    hT = []
    for c in range(NCH):
        hT.append(const.tile([128, S], BF16, name=f"hT{c}", tag=f"hT{c}"))
        nc.gpsimd.dma_start(out=hT[c][:, :], in_=hTd[128 * c:128 * (c + 1), :])

    # persistent SBUF tiles
    qaQ = const.tile([128, S], BF16, name="qaQ", tag="qaQ")    # q0 q1 q2
    qaQ2 = const.tile([128, S], BF16, name="qaQ2", tag="qaQ2")  # q3 q4 q5
    kaK = const.tile([128, S], BF16, name="kaK", tag="kaK")    # k0 k1 k2
    kaK2 = const.tile([128, S], BF16, name="kaK2", tag="kaK2")  # k3 k4 k5
    ksqK = const.tile([128, S], BF16, name="ksqK", tag="ksqK")
    ksqK2 = const.tile([128, S], BF16, name="ksqK2", tag="ksqK2")
    ckkT = const.tile([128, 48], F32, name="ckkT", tag="ckkT")
    ckkT2 = const.tile([128, 48], F32, name="ckkT2", tag="ckkT2")
    vsb = [const.tile([128, HPC * (D + 1)], BF16, name=f"v{t}", tag=f"v{t}")
           for t in range(8)]
    ctxn = [const.tile([128, HPC * D], BF16, name=f"ctxn{s}", tag=f"ctxn{s}")
            for s in range(8)]
    ctxT = [const.tile([128, S], BF16, name=f"ctxT{j}", tag=f"ctxT{j}")
            for j in range(3)]
    # preset the ones columns of vsb (denominator trick)
    for t in range(8):
        vv = vsb[t][:, :].rearrange("p (h c) -> p h c", h=HPC)
        nc.vector.memset(vv[:, :, D:D + 1], 1.0)

    # ---------------- qk projection: K and Q first (critical path) --------
    pcK, pcQ = psA(), psA()
    for c in range(NCH):
        st, sp = (c == 0), (c == NCH - 1)
        for n in range(2):
            sl = slice(512 * n, 512 * (n + 1))
            nc.tensor.matmul(out=pcK[:, sl], lhsT=wK[c],
                             rhs=hT[c][:, sl], start=st, stop=sp)
            nc.tensor.matmul(out=pcQ[:, sl], lhsT=wQ[c],
                             rhs=hT[c][:, sl], start=st, stop=sp)
    nc.scalar.activation(out=kaK[:, :], in_=pcK[:, :], func=AF.Copy)
    cp(qaQ[:, :], pcQ[:, :])

    # kk for heads 0-2 (from K), ckkT
    nc.vector.tensor_mul(ksqK[:, :], kaK[:, :], kaK[:, :])
    pkk = psC()
    for t in range(8):
        tsl = slice(128 * t, 128 * (t + 1))
        nc.tensor.matmul(out=pkk[:, 6 * t:6 * t + 6], lhsT=ksqK[:, tsl],
                         rhs=indK[:, :], start=True, stop=False)
        nc.tensor.matmul(out=pkk[:, 6 * t:6 * t + 6], lhsT=kaK[:, tsl],
                         rhs=bvecK[:, :], start=False, stop=True)
    nc.vector.scalar_tensor_tensor(out=ckkT[:, :], in0=pkk[:, 0:48], scalar=0.0,
                                   in1=maskT[:, :], op0=ALU.bypass, op1=ALU.add)

    def qk2_part():
        """Q2/K2 projection + kk for heads 3-5; runs under exp of head 0."""
        for grp, (wg, dst, eng) in enumerate(
                ((wQ2, qaQ2, "dve"), (wK2, kaK2, "act"))):
            ph = [psC(), psC()]
            for c in range(NCH):
                st, sp = (c == 0), (c == NCH - 1)
                for n in range(2):
                    nc.tensor.matmul(out=ph[n][:, 0:512], lhsT=wg[c],
                                     rhs=hT[c][:, 512 * n:512 * (n + 1)],
                                     start=st, stop=sp)
            for n in range(2):
                sl = slice(512 * n, 512 * (n + 1))
                if eng == "act":
                    nc.scalar.activation(out=dst[:, sl], in_=ph[n][:, 0:512],
                                         func=AF.Copy)
                else:
                    cp(dst[:, sl], ph[n][:, 0:512])
        nc.vector.tensor_mul(ksqK2[:, :], kaK2[:, :], kaK2[:, :])
        pk2 = psC()
        for t in range(8):
            tsl = slice(128 * t, 128 * (t + 1))
            nc.tensor.matmul(out=pk2[:, 6 * t:6 * t + 6], lhsT=ksqK2[:, tsl],
                             rhs=indM[:, :], start=True, stop=False)
            nc.tensor.matmul(out=pk2[:, 6 * t:6 * t + 6], lhsT=kaK2[:, tsl],
                             rhs=bvecM[:, :], start=False, stop=True)
        nc.vector.scalar_tensor_tensor(out=ckkT2[:, :], in0=pk2[:, 0:48],
                                       scalar=0.0, in1=maskT[:, :],
                                       op0=ALU.bypass, op1=ALU.add)

    def qa_ap(h):
        return qaQ if h < 3 else qaQ2

    def ka_ap(h):
        return kaK if h < 3 else kaK2

    def base(h):
        return 32 * h if h < 3 else 32 * (h - 3)

    def ckk_col(h, t):
        src_ = ckkT if h < 3 else ckkT2
        return src_[:, 6 * t + h:6 * t + h + 1]

    # ---------------- per-head pipeline ----------------
    # fillers: units of PE work executed between score matmuls
    fillers = []

    def run_filler():
        if fillers:
            fillers.pop(0)()

    def v_tile(s):
        def f():
            pv = psC()
            for c in range(NCH):
                nc.tensor.matmul(out=pv[:, 0:HPC * D],
                                 lhsT=hT[c][:, 128 * s:128 * (s + 1)],
                                 rhs=wv[c][:, :],
                                 start=(c == 0), stop=(c == NCH - 1))
            vv = vsb[s][:, :].rearrange("p (h c) -> p h c", h=HPC)
            nc.vector.scalar_tensor_tensor(
                out=vv[:, :, 0:D],
                in0=pv[:, 0:HPC * D].rearrange("p (h c) -> p h c", h=HPC),
                scalar=0.0,
                in1=bv_bc[:, :].rearrange("p (h c) -> p h c", h=HPC),
                op0=ALU.bypass, op1=ALU.add)
        return f

    def ctx_quad(h, q):
        def f():
            px = psC()
            for i in range(4):
                s = 4 * q + i
                for t in range(8):
                    nc.tensor.matmul(
                        out=px[:, 65 * i:65 * i + 65],
                        lhsT=ptiles[h % 3][t][:, 128 * s:128 * (s + 1)],
                        rhs=vsb[t][:, (D + 1) * h:(D + 1) * (h + 1)],
                        start=(t == 0), stop=(t == 7))
            rec4 = work.tile([128, 4], F32, name="rec4", tag="rec4")
            pxv = px[:, 0:260].rearrange("p (i c) -> p i c", i=4)
            den = pxv[:, :, D:D + 1].rearrange("p i c -> p (i c)")
            nc.vector.reciprocal(rec4[:, :], den)
            for i in range(4):
                s = 4 * q + i
                nc.vector.tensor_scalar_mul(ctxn[s][:, D * h:D * (h + 1)],
                                            px[:, 65 * i:65 * i + D],
                                            rec4[:, i:i + 1])
        return f

    def transp(j, s):
        def f():
            ct = psC()
            pt = ct[:, :].bitcast(BF16)[:, 0:128]
            nc.tensor.transpose(pt, ctxn[s][:, 128 * j:128 * (j + 1)],
                                ident[:, :])
            cp(ctxT[j][:, 128 * s:128 * (s + 1)], pt)
        return f

    # pts double-buffered across heads
    ptiles = [[ptp.tile([128, S], BF16, name=f"pt{p}_{t}", tag=f"pt{p}_{t}")
               for t in range(8)] for p in range(3)]

    def scores_head(h):
        qa, ka, b = qa_ap(h), ka_ap(h), base(h)
        for t in range(8):
            pc = psA()
            for n in range(2):
                nc.tensor.matmul(
                    out=pc[:, 512 * n:512 * (n + 1)],
                    lhsT=ka[b:b + R, 128 * t:128 * (t + 1)],
                    rhs=qa[b:b + R, 512 * n:512 * (n + 1)],
                    start=True, stop=True)
            nc.scalar.activation(out=ptiles[h % 3][t][:, :], in_=pc[:, :],
                                 func=AF.Exp, bias=ckk_col(h, t), scale=ESC)
            run_filler()
            run_filler()

    scores_head(0)
    qk2_part()
    scores_head(1)
    for s in range(8):
        v_tile(s)()
    for h in range(2, HPC):
        fillers = [ctx_quad(h - 2, 0), ctx_quad(h - 2, 1)]
        if h == 4:
            fillers += [transp(0, s) for s in range(8)]
        scores_head(h)
        while fillers:
            run_filler()
    for hh in (HPC - 2, HPC - 1):
        ctx_quad(hh, 0)()
        ctx_quad(hh, 1)()
    for j in (1, 2):
        for s in range(8):
            transp(j, s)()

    # ---------------- out projection, 2-wave tail ----------------
    for w in range(4):
        pos = [psA() for _ in range(2)]
        for i in range(2):
            s = 2 * w + i
            for n0, nw in ((0, 512), (512, 256)):
                for j in range(3):
                    nc.tensor.matmul(out=pos[i][:, n0:n0 + nw],
                                     lhsT=ctxT[j][:, 128 * s:128 * (s + 1)],
                                     rhs=wo[j][:, n0:n0 + nw],
                                     start=(j == 0), stop=(j == 2))
        for i in range(2):
            s = 2 * w + i
            osb = work.tile([128, E], F32, name="osb", tag="osb", bufs=3)
            nc.vector.scalar_tensor_tensor(
                out=osb[:, :], in0=pos[i][:, 0:E], scalar=0.0,
                in1=bo_bc[:, :], op0=ALU.bypass, op1=ALU.add)
            nc.sync.dma_start(out=outd[128 * s:128 * (s + 1), :], in_=osb[:, :])

    stack.close()


_NC_CACHE = None


def _build():
    global _NC_CACHE
    if _NC_CACHE is None:
        nc = bacc.Bacc("TRN2", target_bir_lowering=False, debug=False,
                       enable_asserts=True, num_devices=NCORES)
        with tile.TileContext(nc) as tc:
            _emit(tc)
        nc.compile()
        _NC_CACHE = nc
    return _NC_CACHE


def kernel(hidden_states, attention_mask, Wq, bq, Wk, bk, Wv, bv, Wo, bo, A,
           **_ignored):
    global LAST_RESULTS
    hidden_states = np.asarray(hidden_states, np.float32)
    attention_mask = np.asarray(attention_mask, np.float32)
    Wq, bq = np.asarray(Wq, np.float32), np.asarray(bq, np.float32)
    Wk, bk = np.asarray(Wk, np.float32), np.asarray(bk, np.float32)
    Wv, bv = np.asarray(Wv, np.float32), np.asarray(bv, np.float32)
    Wo, bo = np.asarray(Wo, np.float32), np.asarray(bo, np.float32)
    A = np.asarray(A, np.float32)

    B = hidden_states.shape[0]
    nc = _build()

    bf = ml_dtypes.bfloat16
    ident = np.eye(128, dtype=np.float32)
    in_maps = []
    for c in range(NCORES):
        b = c // 2
        h0 = HPC * (c % 2)
        sl = slice(h0 * D, (h0 + HPC) * D)

        # host-folded Weff = A^T W (per head), 32-col spacing, pads zero
        weffQ = np.zeros((E, 128), np.float32)
        weffK = np.zeros((E, 128), np.float32)
        weffQ2 = np.zeros((E, 128), np.float32)
        weffK2 = np.zeros((E, 128), np.float32)
        bvecK = np.zeros((128, 6), np.float32)
        bvecM2 = np.zeros((128, 6), np.float32)
        indKm = np.zeros((128, 6), np.float32)
        indM2 = np.zeros((128, 6), np.float32)
        for h in range(HPC):
            Ah = A[h0 + h]                                  # (64, 16)
            hd = slice((h0 + h) * D, (h0 + h + 1) * D)
            AtWq = Ah.T @ Wq[hd]                            # (16, 768)
            AtWk = Ah.T @ Wk[hd]
            bqA = Ah.T @ bq[hd]                             # (16,)
            bkA = Ah.T @ bk[hd]
            bvec = ESC * (bqA - bkA)
            if h < 3:
                weffQ[:, 32 * h:32 * h + R] = AtWq.T
                weffK[:, 32 * h:32 * h + R] = AtWk.T
                bvecK[32 * h:32 * h + R, h] = bvec
                indKm[32 * h:32 * h + 32, h] = -SCALE
            else:
                g = h - 3
                weffQ2[:, 32 * g:32 * g + R] = AtWq.T
                weffK2[:, 32 * g:32 * g + R] = AtWk.T
                bvecM2[32 * g:32 * g + R, h] = bvec
                indM2[32 * g:32 * g + 32, h] = -SCALE

        maskT48 = np.repeat(
            attention_mask[b, 0, 0].reshape(8, 128).T, 6, axis=1)  # (128, 48)

        def wpack(w):
            # (768, 128) -> (128, 6*128): out[p, 128c+m] = w[128c+p, m]
            return np.ascontiguousarray(
                w.reshape(NCH, 128, 128).transpose(1, 0, 2).reshape(128, -1)
                .astype(bf))

        in_maps.append({
            "hTa": np.ascontiguousarray(hidden_states[b].T.astype(bf)),
            "weffQ": wpack(weffQ),
            "weffK": wpack(weffK),
            "weffQ2": wpack(weffQ2),
            "weffK2": wpack(weffK2),
            "WvTa": np.ascontiguousarray(Wv[sl].T.astype(bf)),
            "WoT": np.ascontiguousarray(Wo[:, sl].T.astype(bf)),
            "maskT48": np.ascontiguousarray(maskT48.astype(np.float32)),
            "indK": np.ascontiguousarray(indKm.astype(bf)),
            "bvecK": np.ascontiguousarray(bvecK.astype(bf)),
            "indM2": np.ascontiguousarray(indM2.astype(bf)),
            "bvecM2": np.ascontiguousarray(bvecM2.astype(bf)),
            "bv_bc": np.ascontiguousarray(
                np.broadcast_to(bv[sl], (128, HPC * D)).astype(np.float32)),
            "bo2": np.ascontiguousarray(
                np.broadcast_to(bo / 2.0, (128, E)).astype(np.float32)),
            "ident": np.ascontiguousarray(ident.astype(bf)),
        })

    res = run_bass_kernel_spmd(nc, in_maps, list(range(NCORES)),
                               trace=bool(os.environ.get("KERNEL_TRACE")))
    LAST_RESULTS = res
    parts = [res.results[c]["outp"] for c in range(NCORES)]
    out = np.stack([parts[2 * b] + parts[2 * b + 1] for b in range(B)], 0)
    return np.ascontiguousarray(out.astype(np.float32))
